# revision 27
# baseline (speedup 1.0000x reference)
"""AnnoCluster (VQ codebook autoencoder) Trainium2 kernel.

Data-parallel across 8 NeuronCores: batch dim of x sharded (512 rows/core),
weights replicated. Host passes x transposed per shard (and split into exact
bf16 hi/lo halves) and the big decoder outputs come back transposed; all
on-device matmuls then contract along the partition axis with no on-device
transposes of large tensors, and every bias is a per-partition scalar.

Encoder precision: x and enc_w1 are split on the host into bf16 hi + bf16 lo
(lo = round_bf16(x - hi)). h1 = xh@wh + xh@wl + xl@wh accumulated in fp32
PSUM reproduces the fp32 matmul to ~1e-5 relative (the dropped xl@wl term is
O(2^-18)), which keeps the downstream argmax over 16 centroids exact while
running the PE at bf16 speed. Decoders run plain bf16 (output tolerance).
"""

import sys

import numpy as np

if "/opt/trn_rl_repo" not in sys.path:
    sys.path.append("/opt/trn_rl_repo")

import ml_dtypes  # noqa: E402

import concourse.bass as bass  # noqa: E402
import concourse.tile as tile  # noqa: E402
from concourse import bacc, mybir  # noqa: E402
from concourse.bass_utils import run_bass_kernel_spmd  # noqa: E402

F32 = mybir.dt.float32
F16 = mybir.dt.float16
BF16 = mybir.dt.bfloat16
I32 = mybir.dt.int32
AX = mybir.AxisListType
ALU = mybir.AluOpType
ACTF = mybir.ActivationFunctionType

B, D, H, Z, K = 4096, 10000, 128, 32, 16
NCORES = 8
BL = B // NCORES  # 512 rows per core
DT = 128
D_FULL = D // DT  # 78 full tiles
D_REM = D - D_FULL * DT  # 16
ND = D_FULL + 1  # 79
NB = BL // 128  # 4 batch tiles of 128 rows
T_DF = 10.0


def build_nc():
    nc = bacc.Bacc(None, target_bir_lowering=False)

    xh = nc.dram_tensor("xh", [128, ND, BL], BF16, kind="ExternalInput")
    xl = nc.dram_tensor("xl", [128, ND, BL], BF16, kind="ExternalInput")
    w1h = nc.dram_tensor("w1h", [128, ND, H], BF16, kind="ExternalInput")
    w1l = nc.dram_tensor("w1l", [128, ND, H], BF16, kind="ExternalInput")
    b1 = nc.dram_tensor("b1", [H, 1], F32, kind="ExternalInput")
    w2 = nc.dram_tensor("w2", [H, Z], F32, kind="ExternalInput")
    b2 = nc.dram_tensor("b2", [Z, 1], F32, kind="ExternalInput")
    emb = nc.dram_tensor("emb", [K, Z], F32, kind="ExternalInput")
    dl = nc.dram_tensor("dl", [Z + 1, K], F32, kind="ExternalInput")
    we1 = nc.dram_tensor("we1", [Z, H], F32, kind="ExternalInput")
    be1 = nc.dram_tensor("be1", [H, 1], F32, kind="ExternalInput")
    we2 = nc.dram_tensor("we2", [H, D], BF16, kind="ExternalInput")
    be2t = nc.dram_tensor("be2t", [DT, ND], F32, kind="ExternalInput")
    wq1 = nc.dram_tensor("wq1", [Z, H], F32, kind="ExternalInput")
    bq1 = nc.dram_tensor("bq1", [H, 1], F32, kind="ExternalInput")
    wq2 = nc.dram_tensor("wq2", [H, D], BF16, kind="ExternalInput")
    bq2t = nc.dram_tensor("bq2t", [DT, ND], F32, kind="ExternalInput")
    iota4 = nc.dram_tensor("iota4", [128, NB * K], F32, kind="ExternalInput")
    desc4 = nc.dram_tensor("desc4", [128, NB * K], F32, kind="ExternalInput")
    ident = nc.dram_tensor("ident", [128, 128], F32, kind="ExternalInput")

    xeT = nc.dram_tensor("xeT", [D, BL], F16, kind="ExternalOutput")
    xqT = nc.dram_tensor("xqT", [D, BL], F16, kind="ExternalOutput")
    ze = nc.dram_tensor("ze", [BL, Z], F32, kind="ExternalOutput")
    zq = nc.dram_tensor("zq", [BL, Z], F32, kind="ExternalOutput")
    ko = nc.dram_tensor("ko", [BL, 1], I32, kind="ExternalOutput")
    zd = nc.dram_tensor("zd", [BL, K], F32, kind="ExternalOutput")
    dp = nc.dram_tensor("dp", [BL, K], F32, kind="ExternalOutput")

    with tile.TileContext(nc) as tc:
        with (
            tc.tile_pool(name="const", bufs=1) as constp,
            tc.tile_pool(name="wbig", bufs=1) as wbig,
            tc.tile_pool(name="xin", bufs=3) as xin,
            tc.tile_pool(name="win", bufs=3) as win,
            tc.tile_pool(name="mid", bufs=1) as mid,
            tc.tile_pool(name="small", bufs=2) as small,
            tc.tile_pool(name="outb", bufs=12) as outb,
            tc.tile_pool(name="ps_big", bufs=6, space="PSUM") as ps_big,
            tc.tile_pool(name="ps_oh", bufs=1, space="PSUM") as ps_oh,
            tc.tile_pool(name="ps_small", bufs=1, space="PSUM") as ps_small,
        ):
            # ---- encoder: h1[H, BL] = relu(w1.T @ x + b1), split-bf16 exact ----
            # x/w1 arrive zero-padded to 79*128 rows in partition-major tiled
            # layout, so every DMA moves G k-tiles with multi-KB contiguous
            # runs per partition (cheap descriptor generation).
            h1_ps = ps_big.tile([H, BL], F32, tag="out")
            G = 8
            bounds = [0, 2, 8]
            while bounds[-1] < ND:
                bounds.append(min(bounds[-1] + G, ND))
            n_enc_mm = 3 * ND
            mm_i = 0
            for g in range(len(bounds) - 1):
                g0, g1 = bounds[g], bounds[g + 1]
                gn = g1 - g0
                gsl = slice(g0, g1)
                wht = win.tile([128, G, H], BF16, tag="wht")
                nc.gpsimd.dma_start(out=wht[:, :gn, :], in_=w1h[:, gsl, :])
                wlt = win.tile([128, G, H], BF16, tag="wlt")
                nc.gpsimd.dma_start(out=wlt[:, :gn, :], in_=w1l[:, gsl, :])
                xht = xin.tile([128, G, BL], BF16, tag="xht")
                nc.sync.dma_start(out=xht[:, :gn, :], in_=xh[:, gsl, :])
                xlt = xin.tile([128, G, BL], BF16, tag="xlt")
                nc.sync.dma_start(out=xlt[:, :gn, :], in_=xl[:, gsl, :])
                for u in range(gn):
                    for lhs_t, rhs_t in (
                        (wht[:, u, :], xht[:, u, :]),
                        (wht[:, u, :], xlt[:, u, :]),
                        (wlt[:, u, :], xht[:, u, :]),
                    ):
                        nc.tensor.matmul(
                            h1_ps, lhsT=lhs_t, rhs=rhs_t,
                            start=(mm_i == 0), stop=(mm_i == n_enc_mm - 1),
                        )
                        mm_i += 1

            # ---- constants / decoder weights (issued late, overlap encoder) ----
            b1_sb = constp.tile([H, 1], F32)
            nc.scalar.dma_start(out=b1_sb, in_=b1[:, :])
            w2_sb = constp.tile([H, Z], F32)
            nc.scalar.dma_start(out=w2_sb, in_=w2[:, :])
            b2_sb = constp.tile([Z, 1], F32)
            nc.scalar.dma_start(out=b2_sb, in_=b2[:, :])
            emb_sb = constp.tile([K, Z], F32)
            nc.scalar.dma_start(out=emb_sb, in_=emb[:, :])
            dl_sb = constp.tile([Z + 1, K], F32)
            nc.scalar.dma_start(out=dl_sb, in_=dl[:, :])
            we1_sb = constp.tile([Z, H], F32)
            nc.scalar.dma_start(out=we1_sb, in_=we1[:, :])
            be1_sb = constp.tile([H, 1], F32)
            nc.scalar.dma_start(out=be1_sb, in_=be1[:, :])
            be2t_sb = constp.tile([DT, ND], F32)
            nc.scalar.dma_start(out=be2t_sb, in_=be2t[:, :])
            wq1_sb = constp.tile([Z, H], F32)
            nc.scalar.dma_start(out=wq1_sb, in_=wq1[:, :])
            bq1_sb = constp.tile([H, 1], F32)
            nc.scalar.dma_start(out=bq1_sb, in_=bq1[:, :])
            bq2t_sb = constp.tile([DT, ND], F32)
            nc.scalar.dma_start(out=bq2t_sb, in_=bq2t[:, :])
            iota_sb = constp.tile([128, NB, K], F32)
            nc.scalar.dma_start(
                out=iota_sb, in_=iota4[:, :].rearrange("p (i k) -> p i k", k=K)
            )
            desc_sb = constp.tile([128, NB, K], F32)
            nc.scalar.dma_start(
                out=desc_sb, in_=desc4[:, :].rearrange("p (i k) -> p i k", k=K)
            )
            ident_sb = constp.tile([128, 128], F32)
            nc.scalar.dma_start(out=ident_sb, in_=ident[:, :])

            h1_sb = mid.tile([H, BL], F32)
            nc.scalar.activation(
                out=h1_sb, in_=h1_ps, func=ACTF.Relu, bias=b1_sb, scale=1.0
            )
            we2_sb = wbig.tile([H, D], BF16)
            nc.scalar.dma_start(out=we2_sb, in_=we2[:, :])
            wq2_sb = wbig.tile([H, D], BF16)
            nc.scalar.dma_start(out=wq2_sb, in_=wq2[:, :])

            # ---- z_eT[Z, BL] (+b2) into aug rows 0..Z-1; row Z = ones ----
            aug_sb = mid.tile([Z + 1, BL], F32)
            ze_ps = ps_small.tile([Z, BL], F32, tag="sm")
            nc.tensor.matmul(ze_ps, lhsT=w2_sb, rhs=h1_sb, start=True, stop=True)
            nc.scalar.activation(
                out=aug_sb[0:Z, :], in_=ze_ps, func=ACTF.Identity, bias=b2_sb,
                scale=1.0,
            )
            nc.vector.memset(aug_sb[Z : Z + 1, :], 1.0)

            # ---- decoder-e hidden early, then weave the first xe tiles in
            # front of the argmax chain so PE and out-DMA stay busy ----
            he_ps = ps_big.tile([H, BL], F32, tag="out")
            nc.tensor.matmul(
                he_ps, lhsT=we1_sb, rhs=aug_sb[0:Z, :], start=True, stop=True
            )
            he_sb = mid.tile([H, BL], BF16)
            nc.scalar.activation(
                out=he_sb, in_=he_ps, func=ACTF.Relu, bias=be1_sb, scale=1.0
            )

            def out_tile(m, w_sb, bias_sb, dst, use_act, rhs_sb, tag):
                dm = DT if m < D_FULL else D_REM
                dsl = slice(m * DT, m * DT + dm)
                o_ps = ps_big.tile([DT, BL], F32, tag="out")
                nc.tensor.matmul(
                    o_ps[:dm], lhsT=w_sb[:, dsl], rhs=rhs_sb, start=True, stop=True
                )
                o_sb = outb.tile([DT, BL], F16, tag=tag)
                if use_act:
                    nc.scalar.activation(
                        out=o_sb[:dm], in_=o_ps[:dm], func=ACTF.Identity,
                        bias=bias_sb[:dm, m : m + 1], scale=1.0,
                    )
                    nc.gpsimd.dma_start(out=dst[dsl, :], in_=o_sb[:dm])
                else:
                    nc.vector.tensor_scalar(
                        out=o_sb[:dm], in0=o_ps[:dm],
                        scalar1=bias_sb[:dm, m : m + 1], scalar2=None, op0=ALU.add,
                    )
                    nc.sync.dma_start(out=dst[dsl, :], in_=o_sb[:dm])

            def xe_tile(m, use_act):
                out_tile(m, we2_sb, be2t_sb, xeT, use_act, he_sb, "xeo")

            def xq_tile(m, use_act):
                out_tile(m, wq2_sb, bq2t_sb, xqT, use_act, hq_sb, "xqo")

            # Weave decoder-e tiles between the argmax-chain PE op groups so
            # the PE and the output DMA never drain while the cross-engine
            # argmax chain resolves (it is latency- not throughput-bound).
            xe_cur = [0]

            def emit_xe(n):
                for _ in range(n):
                    m = xe_cur[0]
                    xe_tile(m, use_act=(m % 2 == 1))
                    xe_cur[0] += 1

            emit_xe(2)

            # ---- z_distT[K, BL] = -2*emb@z_e + |emb|^2 (|z_e|^2 added later) ----
            zdT_ps = ps_small.tile([K, BL], F32, tag="sm")
            nc.tensor.matmul(zdT_ps, lhsT=dl_sb, rhs=aug_sb, start=True, stop=True)
            zdT_sb = mid.tile([K, BL], F32)
            nc.vector.tensor_copy(out=zdT_sb, in_=zdT_ps)

            emit_xe(2)

            # ---- batched z block: all 4 row-tiles as [128, NB, *] tensors ----
            zet_ps = ps_small.tile([128, NB, Z], F32, tag="sm")
            for i in range(NB):
                nc.tensor.transpose(
                    zet_ps[:, i, :],
                    in_=aug_sb[0:Z, i * 128 : (i + 1) * 128],
                    identity=ident_sb[0:Z, 0:Z],
                )
            ze_all = small.tile([128, NB, Z], F32, tag="zeall")
            nc.vector.tensor_copy(out=ze_all, in_=zet_ps)
            nc.sync.dma_start(
                out=ze[:, :].rearrange("(i p) z -> p i z", p=128), in_=ze_all
            )
            zesq = small.tile([128, NB, Z], F32, tag="zesq")
            nc.vector.tensor_mul(zesq, ze_all, ze_all)
            ss_all = small.tile([128, NB], F32, tag="ss")
            nc.vector.reduce_sum(out=ss_all, in_=zesq, axis=AX.X)

            emit_xe(3)

            zdt_ps = ps_small.tile([128, NB, K], F32, tag="sm")
            for i in range(NB):
                nc.tensor.transpose(
                    zdt_ps[:, i, :],
                    in_=zdT_sb[:, i * 128 : (i + 1) * 128],
                    identity=ident_sb[0:K, 0:K],
                )
            zd_all = small.tile([128, NB, K], F32, tag="zdall")
            nc.vector.tensor_tensor(
                out=zd_all, in0=zdt_ps,
                in1=ss_all[:, :].broadcast_to([128, NB, K]),
                op=ALU.add,
            )
            nc.sync.dma_start(
                out=zd[:, :].rearrange("(i p) k -> p i k", p=128), in_=zd_all
            )

            emit_xe(3)

            # dist_prob = (1 + d/T_DF) ** -(T_DF+1)/2, row-normalized
            t1 = small.tile([128, NB, K], F32, tag="t1")
            nc.scalar.activation(
                out=t1, in_=zd_all, func=ACTF.Ln, bias=1.0, scale=1.0 / T_DF
            )
            p_all = small.tile([128, NB, K], F32, tag="pall")
            nc.scalar.activation(
                out=p_all, in_=t1, func=ACTF.Exp, bias=0.0, scale=-(T_DF + 1.0) / 2.0
            )
            s_all = small.tile([128, NB], F32, tag="sall")
            nc.vector.reduce_sum(out=s_all, in_=p_all, axis=AX.X)
            rs_all = small.tile([128, NB], F32, tag="rsall")
            nc.vector.reciprocal(out=rs_all, in_=s_all)
            pn_all = small.tile([128, NB, K], F32, tag="pnall")
            nc.vector.tensor_tensor(
                out=pn_all, in0=p_all,
                in1=rs_all[:, :].broadcast_to([128, NB, K]),
                op=ALU.mult,
            )
            nc.sync.dma_start(
                out=dp[:, :].rearrange("(i p) k -> p i k", p=128), in_=pn_all
            )

            emit_xe(4)

            # argmax (first max wins): k = 15 - max((15 - j) * (pn == max))
            mx_all = small.tile([128, NB], F32, tag="mxall")
            nc.vector.reduce_max(out=mx_all, in_=pn_all, axis=AX.X)
            eq_all = small.tile([128, NB, K], F32, tag="eqall")
            nc.vector.tensor_tensor(
                out=eq_all, in0=pn_all,
                in1=mx_all[:, :].broadcast_to([128, NB, K]),
                op=ALU.is_equal,
            )
            t2_all = small.tile([128, NB, K], F32, tag="t2all")
            nc.vector.tensor_mul(t2_all, eq_all, desc_sb)
            rm_all = small.tile([128, NB], F32, tag="rmall")
            nc.vector.reduce_max(out=rm_all, in_=t2_all, axis=AX.X)
            kf_all = small.tile([128, NB], F32, tag="kfall")
            nc.vector.tensor_scalar(
                out=kf_all, in0=rm_all, scalar1=-1.0, scalar2=float(K - 1),
                op0=ALU.mult, op1=ALU.add,
            )
            ki_all = small.tile([128, NB], I32, tag="kiall")
            nc.vector.tensor_copy(out=ki_all, in_=kf_all)
            nc.sync.dma_start(
                out=ko[:, :].rearrange("(i p) o -> p i o", p=128),
                in_=ki_all[:, :].broadcast_to([128, NB, 1]),
            )
            oh_all = small.tile([128, NB, K], F32, tag="ohall")
            nc.vector.tensor_tensor(
                out=oh_all, in0=iota_sb,
                in1=kf_all[:, :].broadcast_to([128, NB, K]),
                op=ALU.is_equal,
            )

            emit_xe(4)

            ohT_ps = ps_oh.tile([K, BL], F32)
            for i in range(NB):
                nc.tensor.transpose(
                    ohT_ps[:, i * 128 : (i + 1) * 128], in_=oh_all[:, i, :],
                    identity=ident_sb,
                )
            ohT_sb = mid.tile([K, BL], F32)
            nc.vector.tensor_copy(out=ohT_sb, in_=ohT_ps)

            emit_xe(3)

            # ---- z_q: zqT[Z, BL] = emb.T @ onehotT; zq rows out ----
            zqT_ps = ps_small.tile([Z, BL], F32, tag="sm")
            nc.tensor.matmul(zqT_ps, lhsT=emb_sb, rhs=ohT_sb, start=True, stop=True)
            zqT_sb = mid.tile([Z, BL], F32)
            nc.vector.tensor_copy(out=zqT_sb, in_=zqT_ps)
            zq_ps = ps_small.tile([128, NB, Z], F32, tag="sm")
            for i in range(NB):
                nc.tensor.matmul(
                    zq_ps[:, i, :], lhsT=ohT_sb[:, i * 128 : (i + 1) * 128],
                    rhs=emb_sb, start=True, stop=True,
                )
            zq_all = small.tile([128, NB, Z], F32, tag="zqall")
            nc.vector.tensor_copy(out=zq_all, in_=zq_ps)
            nc.sync.dma_start(
                out=zq[:, :].rearrange("(i p) z -> p i z", p=128), in_=zq_all
            )

            emit_xe(3)

            # ---- decoder-q hidden ----
            hq_ps = ps_big.tile([H, BL], F32, tag="out")
            nc.tensor.matmul(hq_ps, lhsT=wq1_sb, rhs=zqT_sb, start=True, stop=True)
            hq_sb = mid.tile([H, BL], BF16)
            nc.scalar.activation(
                out=hq_sb, in_=hq_ps, func=ACTF.Relu, bias=bq1_sb, scale=1.0
            )

            # ---- remaining decoder tiles: spread the leftover xe tiles
            # evenly across the xq stream so both streams (and both evict
            # engines) stay active until the very last tile ----
            n_early = xe_cur[0]
            n_rest = ND - n_early
            sent = 0
            for j in range(ND):
                target = (j + 1) * n_rest // ND
                while sent < target:
                    m = n_early + sent
                    xe_tile(m, use_act=(m % 2 == 1))
                    sent += 1
                xq_tile(j, use_act=(j % 2 == 0))

    nc.compile()
    return nc


def _pad_bias_t(b):
    """[D] bias -> [DT, ND] where column m is b[m*DT : m*DT+DT] (zero padded)."""
    bp = np.zeros(ND * DT, dtype=np.float32)
    bp[:D] = b
    return np.ascontiguousarray(bp.reshape(ND, DT).T)


def _split_bf16(a):
    """Exact-ish split: a ~= hi + lo with both bf16 (lo holds the residual)."""
    hi = a.astype(ml_dtypes.bfloat16)
    lo = (a - hi.astype(np.float32)).astype(ml_dtypes.bfloat16)
    return np.ascontiguousarray(hi), np.ascontiguousarray(lo)


def _tile_pm(a):
    """[Drows, C] -> zero-pad rows to ND*128 -> partition-major [128, ND, C]."""
    rows, c = a.shape
    out = np.zeros((ND * 128, c), dtype=a.dtype)
    out[:rows] = a
    return np.ascontiguousarray(out.reshape(ND, 128, c).transpose(1, 0, 2))


def _prep_shared(inputs):
    emb = np.asarray(inputs["embeddings"], dtype=np.float32)
    dl = np.concatenate(
        [-2.0 * emb.T, (emb * emb).sum(axis=1, dtype=np.float32)[None, :]], axis=0
    ).astype(np.float32)
    iota = np.tile(np.arange(K, dtype=np.float32), (128, NB))
    w1h, w1l = _split_bf16(np.asarray(inputs["enc_w1"], np.float32))
    shared = {
        "w1h": _tile_pm(w1h),
        "w1l": _tile_pm(w1l),
        "b1": np.asarray(inputs["enc_b1"], np.float32).reshape(H, 1),
        "w2": np.ascontiguousarray(inputs["enc_w2"], dtype=np.float32),
        "b2": np.asarray(inputs["enc_b2"], np.float32).reshape(Z, 1),
        "emb": np.ascontiguousarray(emb),
        "dl": np.ascontiguousarray(dl),
        "we1": np.ascontiguousarray(inputs["dec_e_w1"], dtype=np.float32),
        "be1": np.asarray(inputs["dec_e_b1"], np.float32).reshape(H, 1),
        "we2": np.ascontiguousarray(
            np.asarray(inputs["dec_e_w2"], np.float32).astype(ml_dtypes.bfloat16)
        ),
        "be2t": _pad_bias_t(np.asarray(inputs["dec_e_b2"], np.float32)),
        "wq1": np.ascontiguousarray(inputs["dec_q_w1"], dtype=np.float32),
        "bq1": np.asarray(inputs["dec_q_b1"], np.float32).reshape(H, 1),
        "wq2": np.ascontiguousarray(
            np.asarray(inputs["dec_q_w2"], np.float32).astype(ml_dtypes.bfloat16)
        ),
        "bq2t": _pad_bias_t(np.asarray(inputs["dec_q_b2"], np.float32)),
        "iota4": np.ascontiguousarray(iota),
        "desc4": np.ascontiguousarray(float(K - 1) - iota),
        "ident": np.eye(128, dtype=np.float32),
    }
    return shared


_NC_CACHE = None


def _run(inputs, trace=False, **kwargs):
    global _NC_CACHE
    if _NC_CACHE is None:
        _NC_CACHE = build_nc()
    nc = _NC_CACHE

    x = np.asarray(inputs["x"], dtype=np.float32)
    shared = _prep_shared(inputs)
    in_maps = []
    for c in range(NCORES):
        m = dict(shared)
        xt = np.ascontiguousarray(x[c * BL : (c + 1) * BL, :].T)
        xth, xtl = _split_bf16(xt)
        m["xh"], m["xl"] = _tile_pm(xth), _tile_pm(xtl)
        in_maps.append(m)

    try:
        res = run_bass_kernel_spmd(
            nc, in_maps, core_ids=list(range(NCORES)), trace=trace, **kwargs
        )
    except Exception:
        # transient NRT/device hiccups occasionally fail a run; retry once
        import time as _time

        _time.sleep(5)
        res = run_bass_kernel_spmd(
            nc, in_maps, core_ids=list(range(NCORES)), trace=trace, **kwargs
        )
    outs = res.results

    x_e = np.ascontiguousarray(
        np.concatenate([o["xeT"].T.astype(np.float32) for o in outs], axis=0), dtype=np.float32
    )
    x_q = np.ascontiguousarray(
        np.concatenate([o["xqT"].T.astype(np.float32) for o in outs], axis=0), dtype=np.float32
    )
    z_e = np.concatenate([o["ze"] for o in outs], axis=0)
    z_q = np.concatenate([o["zq"] for o in outs], axis=0)
    k = np.concatenate([o["ko"][:, 0] for o in outs], axis=0).astype(np.int32)
    z_dist = np.concatenate([o["zd"] for o in outs], axis=0)
    dist_prob = np.concatenate([o["dp"] for o in outs], axis=0)
    return (x_e, x_q, z_e, z_q, k, z_dist, dist_prob), res


def kernel(**inputs):
    out, _ = _run(inputs, trace=False)
    return out


# revision 28
# speedup vs baseline: 1.1184x; 1.1184x over previous
"""AnnoCluster (VQ codebook autoencoder) Trainium2 kernel.

Data-parallel across 8 NeuronCores: batch dim of x sharded (512 rows/core),
weights replicated. Host passes x transposed per shard (and split into exact
bf16 hi/lo halves) and the big decoder outputs come back transposed; all
on-device matmuls then contract along the partition axis with no on-device
transposes of large tensors, and every bias is a per-partition scalar.

Encoder precision: x and enc_w1 are split on the host into bf16 hi + bf16 lo
(lo = round_bf16(x - hi)). h1 = xh@wh + xh@wl + xl@wh accumulated in fp32
PSUM reproduces the fp32 matmul to ~1e-5 relative (the dropped xl@wl term is
O(2^-18)), which keeps the downstream argmax over 16 centroids exact while
running the PE at bf16 speed. Decoders run plain bf16 (output tolerance).
"""

import sys

import numpy as np

if "/opt/trn_rl_repo" not in sys.path:
    sys.path.append("/opt/trn_rl_repo")

import ml_dtypes  # noqa: E402

import concourse.bass as bass  # noqa: E402
import concourse.tile as tile  # noqa: E402
from concourse import bacc, mybir  # noqa: E402
from concourse.bass_utils import run_bass_kernel_spmd  # noqa: E402

F32 = mybir.dt.float32
F16 = mybir.dt.float16
BF16 = mybir.dt.bfloat16
I32 = mybir.dt.int32
AX = mybir.AxisListType
ALU = mybir.AluOpType
ACTF = mybir.ActivationFunctionType

B, D, H, Z, K = 4096, 10000, 128, 32, 16
NCORES = 8
BL = B // NCORES  # 512 rows per core
DT = 128
D_FULL = D // DT  # 78 full tiles
D_REM = D - D_FULL * DT  # 16
ND = D_FULL + 1  # 79
NB = BL // 128  # 4 batch tiles of 128 rows
T_DF = 10.0


def build_nc():
    nc = bacc.Bacc(None, target_bir_lowering=False)

    xh = nc.dram_tensor("xh", [128, ND, BL], BF16, kind="ExternalInput")
    xl = nc.dram_tensor("xl", [128, ND, BL], BF16, kind="ExternalInput")
    w1h = nc.dram_tensor("w1h", [128, ND, H], BF16, kind="ExternalInput")
    w1l = nc.dram_tensor("w1l", [128, ND, H], BF16, kind="ExternalInput")
    b1 = nc.dram_tensor("b1", [H, 1], F32, kind="ExternalInput")
    w2 = nc.dram_tensor("w2", [H, Z], F32, kind="ExternalInput")
    b2 = nc.dram_tensor("b2", [Z, 1], F32, kind="ExternalInput")
    emb = nc.dram_tensor("emb", [K, Z], F32, kind="ExternalInput")
    dl = nc.dram_tensor("dl", [Z + 1, K], F32, kind="ExternalInput")
    we1 = nc.dram_tensor("we1", [Z, H], F32, kind="ExternalInput")
    be1 = nc.dram_tensor("be1", [H, 1], F32, kind="ExternalInput")
    we2 = nc.dram_tensor("we2", [H, D], BF16, kind="ExternalInput")
    be2t = nc.dram_tensor("be2t", [DT, ND], F32, kind="ExternalInput")
    wq1 = nc.dram_tensor("wq1", [Z, H], F32, kind="ExternalInput")
    bq1 = nc.dram_tensor("bq1", [H, 1], F32, kind="ExternalInput")
    wq2 = nc.dram_tensor("wq2", [H, D], BF16, kind="ExternalInput")
    bq2t = nc.dram_tensor("bq2t", [DT, ND], F32, kind="ExternalInput")
    iota4 = nc.dram_tensor("iota4", [128, NB * K], F32, kind="ExternalInput")
    desc4 = nc.dram_tensor("desc4", [128, NB * K], F32, kind="ExternalInput")
    ident = nc.dram_tensor("ident", [128, 128], F32, kind="ExternalInput")

    xeT = nc.dram_tensor("xeT", [D, BL], F16, kind="ExternalOutput")
    xqT = nc.dram_tensor("xqT", [D, BL], F16, kind="ExternalOutput")
    ze = nc.dram_tensor("ze", [BL, Z], F32, kind="ExternalOutput")
    zq = nc.dram_tensor("zq", [BL, Z], F32, kind="ExternalOutput")
    ko = nc.dram_tensor("ko", [BL, 1], I32, kind="ExternalOutput")
    zd = nc.dram_tensor("zd", [BL, K], F32, kind="ExternalOutput")
    dp = nc.dram_tensor("dp", [BL, K], F32, kind="ExternalOutput")

    with tile.TileContext(nc) as tc:
        with (
            tc.tile_pool(name="const", bufs=1) as constp,
            tc.tile_pool(name="wbig", bufs=1) as wbig,
            tc.tile_pool(name="xin", bufs=3) as xin,
            tc.tile_pool(name="win", bufs=3) as win,
            tc.tile_pool(name="mid", bufs=1) as mid,
            tc.tile_pool(name="small", bufs=2) as small,
            tc.tile_pool(name="outb", bufs=12) as outb,
            tc.tile_pool(name="ps_big", bufs=6, space="PSUM") as ps_big,
            tc.tile_pool(name="ps_oh", bufs=1, space="PSUM") as ps_oh,
            tc.tile_pool(name="ps_small", bufs=1, space="PSUM") as ps_small,
        ):
            # ---- encoder: h1[H, BL] = relu(w1.T @ x + b1), split-bf16 exact ----
            # x/w1 arrive zero-padded to 79*128 rows in partition-major tiled
            # layout, so every DMA moves G k-tiles with multi-KB contiguous
            # runs per partition (cheap descriptor generation).
            h1_ps = ps_big.tile([H, BL], F32, tag="out")
            G = 8
            bounds = [0, 1, 3, 8]
            while bounds[-1] < ND:
                bounds.append(min(bounds[-1] + G, ND))
            n_enc_mm = 3 * ND
            mm_i = 0
            for g in range(len(bounds) - 1):
                g0, g1 = bounds[g], bounds[g + 1]
                gn = g1 - g0
                gsl = slice(g0, g1)
                wht = win.tile([128, G, H], BF16, tag="wht")
                nc.gpsimd.dma_start(out=wht[:, :gn, :], in_=w1h[:, gsl, :])
                wlt = win.tile([128, G, H], BF16, tag="wlt")
                nc.gpsimd.dma_start(out=wlt[:, :gn, :], in_=w1l[:, gsl, :])
                xht = xin.tile([128, G, BL], BF16, tag="xht")
                nc.sync.dma_start(out=xht[:, :gn, :], in_=xh[:, gsl, :])
                xlt = xin.tile([128, G, BL], BF16, tag="xlt")
                nc.sync.dma_start(out=xlt[:, :gn, :], in_=xl[:, gsl, :])
                for u in range(gn):
                    for lhs_t, rhs_t in (
                        (wht[:, u, :], xht[:, u, :]),
                        (wlt[:, u, :], xht[:, u, :]),
                        (wht[:, u, :], xlt[:, u, :]),
                    ):
                        nc.tensor.matmul(
                            h1_ps, lhsT=lhs_t, rhs=rhs_t,
                            start=(mm_i == 0), stop=(mm_i == n_enc_mm - 1),
                        )
                        mm_i += 1

            # ---- constants / decoder weights (issued late, overlap encoder) ----
            b1_sb = constp.tile([H, 1], F32)
            nc.scalar.dma_start(out=b1_sb, in_=b1[:, :])
            w2_sb = constp.tile([H, Z], F32)
            nc.scalar.dma_start(out=w2_sb, in_=w2[:, :])
            b2_sb = constp.tile([Z, 1], F32)
            nc.scalar.dma_start(out=b2_sb, in_=b2[:, :])
            emb_sb = constp.tile([K, Z], F32)
            nc.scalar.dma_start(out=emb_sb, in_=emb[:, :])
            dl_sb = constp.tile([Z + 1, K], F32)
            nc.scalar.dma_start(out=dl_sb, in_=dl[:, :])
            we1_sb = constp.tile([Z, H], F32)
            nc.scalar.dma_start(out=we1_sb, in_=we1[:, :])
            be1_sb = constp.tile([H, 1], F32)
            nc.scalar.dma_start(out=be1_sb, in_=be1[:, :])
            be2t_sb = constp.tile([DT, ND], F32)
            nc.scalar.dma_start(out=be2t_sb, in_=be2t[:, :])
            wq1_sb = constp.tile([Z, H], F32)
            nc.scalar.dma_start(out=wq1_sb, in_=wq1[:, :])
            bq1_sb = constp.tile([H, 1], F32)
            nc.scalar.dma_start(out=bq1_sb, in_=bq1[:, :])
            bq2t_sb = constp.tile([DT, ND], F32)
            nc.scalar.dma_start(out=bq2t_sb, in_=bq2t[:, :])
            iota_sb = constp.tile([128, NB, K], F32)
            nc.scalar.dma_start(
                out=iota_sb, in_=iota4[:, :].rearrange("p (i k) -> p i k", k=K)
            )
            desc_sb = constp.tile([128, NB, K], F32)
            nc.scalar.dma_start(
                out=desc_sb, in_=desc4[:, :].rearrange("p (i k) -> p i k", k=K)
            )
            ident_sb = constp.tile([128, 128], F32)
            nc.scalar.dma_start(out=ident_sb, in_=ident[:, :])

            h1_sb = mid.tile([H, BL], F32)
            nc.scalar.activation(
                out=h1_sb, in_=h1_ps, func=ACTF.Relu, bias=b1_sb, scale=1.0
            )
            we2_sb = wbig.tile([H, D], BF16)
            nc.scalar.dma_start(out=we2_sb, in_=we2[:, :])
            wq2_sb = wbig.tile([H, D], BF16)
            nc.scalar.dma_start(out=wq2_sb, in_=wq2[:, :])

            # ---- z_eT[Z, BL] (+b2) into aug rows 0..Z-1; row Z = ones ----
            aug_sb = mid.tile([Z + 1, BL], F32)
            ze_ps = ps_small.tile([Z, BL], F32, tag="sm")
            nc.tensor.matmul(ze_ps, lhsT=w2_sb, rhs=h1_sb, start=True, stop=True)
            nc.scalar.activation(
                out=aug_sb[0:Z, :], in_=ze_ps, func=ACTF.Identity, bias=b2_sb,
                scale=1.0,
            )
            nc.vector.memset(aug_sb[Z : Z + 1, :], 1.0)

            # ---- decoder-e hidden early, then weave the first xe tiles in
            # front of the argmax chain so PE and out-DMA stay busy ----
            he_ps = ps_big.tile([H, BL], F32, tag="out")
            nc.tensor.matmul(
                he_ps, lhsT=we1_sb, rhs=aug_sb[0:Z, :], start=True, stop=True
            )
            he_sb = mid.tile([H, BL], BF16)
            nc.scalar.activation(
                out=he_sb, in_=he_ps, func=ACTF.Relu, bias=be1_sb, scale=1.0
            )

            def out_tile(m, w_sb, bias_sb, dst, use_act, rhs_sb, tag):
                dm = DT if m < D_FULL else D_REM
                dsl = slice(m * DT, m * DT + dm)
                o_ps = ps_big.tile([DT, BL], F32, tag="out")
                nc.tensor.matmul(
                    o_ps[:dm], lhsT=w_sb[:, dsl], rhs=rhs_sb, start=True, stop=True
                )
                o_sb = outb.tile([DT, BL], F16, tag=tag)
                if use_act:
                    nc.scalar.activation(
                        out=o_sb[:dm], in_=o_ps[:dm], func=ACTF.Identity,
                        bias=bias_sb[:dm, m : m + 1], scale=1.0,
                    )
                    nc.gpsimd.dma_start(out=dst[dsl, :], in_=o_sb[:dm])
                else:
                    nc.vector.tensor_scalar(
                        out=o_sb[:dm], in0=o_ps[:dm],
                        scalar1=bias_sb[:dm, m : m + 1], scalar2=None, op0=ALU.add,
                    )
                    nc.sync.dma_start(out=dst[dsl, :], in_=o_sb[:dm])

            def xe_tile(m, use_act):
                out_tile(m, we2_sb, be2t_sb, xeT, use_act, he_sb, "xeo")

            def xq_tile(m, use_act):
                out_tile(m, wq2_sb, bq2t_sb, xqT, use_act, hq_sb, "xqo")

            # Weave decoder-e tiles between the argmax-chain PE op groups so
            # the PE and the output DMA never drain while the cross-engine
            # argmax chain resolves (it is latency- not throughput-bound).
            xe_cur = [0]

            def emit_xe(n):
                for _ in range(n):
                    m = xe_cur[0]
                    xe_tile(m, use_act=(m % 2 == 1))
                    xe_cur[0] += 1

            emit_xe(2)

            # ---- z_distT[K, BL] = -2*emb@z_e + |emb|^2 (|z_e|^2 added later) ----
            zdT_ps = ps_small.tile([K, BL], F32, tag="sm")
            nc.tensor.matmul(zdT_ps, lhsT=dl_sb, rhs=aug_sb, start=True, stop=True)
            zdT_sb = mid.tile([K, BL], F32)
            nc.vector.tensor_copy(out=zdT_sb, in_=zdT_ps)

            emit_xe(2)

            # ---- batched z block: all 4 row-tiles as [128, NB, *] tensors ----
            zet_ps = ps_small.tile([128, NB, Z], F32, tag="sm")
            for i in range(NB):
                nc.tensor.transpose(
                    zet_ps[:, i, :],
                    in_=aug_sb[0:Z, i * 128 : (i + 1) * 128],
                    identity=ident_sb[0:Z, 0:Z],
                )
            ze_all = small.tile([128, NB, Z], F32, tag="zeall")
            nc.vector.tensor_copy(out=ze_all, in_=zet_ps)
            nc.sync.dma_start(
                out=ze[:, :].rearrange("(i p) z -> p i z", p=128), in_=ze_all
            )
            zesq = small.tile([128, NB, Z], F32, tag="zesq")
            nc.vector.tensor_mul(zesq, ze_all, ze_all)
            ss_all = small.tile([128, NB], F32, tag="ss")
            nc.vector.reduce_sum(out=ss_all, in_=zesq, axis=AX.X)

            emit_xe(3)

            zdt_ps = ps_small.tile([128, NB, K], F32, tag="sm")
            for i in range(NB):
                nc.tensor.transpose(
                    zdt_ps[:, i, :],
                    in_=zdT_sb[:, i * 128 : (i + 1) * 128],
                    identity=ident_sb[0:K, 0:K],
                )
            zd_all = small.tile([128, NB, K], F32, tag="zdall")
            nc.vector.tensor_tensor(
                out=zd_all, in0=zdt_ps,
                in1=ss_all[:, :].broadcast_to([128, NB, K]),
                op=ALU.add,
            )
            nc.sync.dma_start(
                out=zd[:, :].rearrange("(i p) k -> p i k", p=128), in_=zd_all
            )

            emit_xe(3)

            # dist_prob = (1 + d/T_DF) ** -(T_DF+1)/2, row-normalized
            t1 = small.tile([128, NB, K], F32, tag="t1")
            nc.scalar.activation(
                out=t1, in_=zd_all, func=ACTF.Ln, bias=1.0, scale=1.0 / T_DF
            )
            p_all = small.tile([128, NB, K], F32, tag="pall")
            nc.scalar.activation(
                out=p_all, in_=t1, func=ACTF.Exp, bias=0.0, scale=-(T_DF + 1.0) / 2.0
            )
            s_all = small.tile([128, NB], F32, tag="sall")
            nc.vector.reduce_sum(out=s_all, in_=p_all, axis=AX.X)
            rs_all = small.tile([128, NB], F32, tag="rsall")
            nc.vector.reciprocal(out=rs_all, in_=s_all)
            pn_all = small.tile([128, NB, K], F32, tag="pnall")
            nc.vector.tensor_tensor(
                out=pn_all, in0=p_all,
                in1=rs_all[:, :].broadcast_to([128, NB, K]),
                op=ALU.mult,
            )
            nc.sync.dma_start(
                out=dp[:, :].rearrange("(i p) k -> p i k", p=128), in_=pn_all
            )

            emit_xe(4)

            # argmax (first max wins): k = 15 - max((15 - j) * (pn == max))
            mx_all = small.tile([128, NB], F32, tag="mxall")
            nc.vector.reduce_max(out=mx_all, in_=pn_all, axis=AX.X)
            eq_all = small.tile([128, NB, K], F32, tag="eqall")
            nc.vector.tensor_tensor(
                out=eq_all, in0=pn_all,
                in1=mx_all[:, :].broadcast_to([128, NB, K]),
                op=ALU.is_equal,
            )
            t2_all = small.tile([128, NB, K], F32, tag="t2all")
            nc.vector.tensor_mul(t2_all, eq_all, desc_sb)
            rm_all = small.tile([128, NB], F32, tag="rmall")
            nc.vector.reduce_max(out=rm_all, in_=t2_all, axis=AX.X)
            kf_all = small.tile([128, NB], F32, tag="kfall")
            nc.vector.tensor_scalar(
                out=kf_all, in0=rm_all, scalar1=-1.0, scalar2=float(K - 1),
                op0=ALU.mult, op1=ALU.add,
            )
            ki_all = small.tile([128, NB], I32, tag="kiall")
            nc.vector.tensor_copy(out=ki_all, in_=kf_all)
            nc.sync.dma_start(
                out=ko[:, :].rearrange("(i p) o -> p i o", p=128),
                in_=ki_all[:, :].broadcast_to([128, NB, 1]),
            )
            oh_all = small.tile([128, NB, K], F32, tag="ohall")
            nc.vector.tensor_tensor(
                out=oh_all, in0=iota_sb,
                in1=kf_all[:, :].broadcast_to([128, NB, K]),
                op=ALU.is_equal,
            )

            emit_xe(4)

            ohT_ps = ps_oh.tile([K, BL], F32)
            for i in range(NB):
                nc.tensor.transpose(
                    ohT_ps[:, i * 128 : (i + 1) * 128], in_=oh_all[:, i, :],
                    identity=ident_sb,
                )
            ohT_sb = mid.tile([K, BL], F32)
            nc.vector.tensor_copy(out=ohT_sb, in_=ohT_ps)

            emit_xe(3)

            # ---- z_q: zqT[Z, BL] = emb.T @ onehotT; zq rows out ----
            zqT_ps = ps_small.tile([Z, BL], F32, tag="sm")
            nc.tensor.matmul(zqT_ps, lhsT=emb_sb, rhs=ohT_sb, start=True, stop=True)
            zqT_sb = mid.tile([Z, BL], F32)
            nc.vector.tensor_copy(out=zqT_sb, in_=zqT_ps)
            zq_ps = ps_small.tile([128, NB, Z], F32, tag="sm")
            for i in range(NB):
                nc.tensor.matmul(
                    zq_ps[:, i, :], lhsT=ohT_sb[:, i * 128 : (i + 1) * 128],
                    rhs=emb_sb, start=True, stop=True,
                )
            zq_all = small.tile([128, NB, Z], F32, tag="zqall")
            nc.vector.tensor_copy(out=zq_all, in_=zq_ps)
            nc.sync.dma_start(
                out=zq[:, :].rearrange("(i p) z -> p i z", p=128), in_=zq_all
            )

            emit_xe(3)

            # ---- decoder-q hidden ----
            hq_ps = ps_big.tile([H, BL], F32, tag="out")
            nc.tensor.matmul(hq_ps, lhsT=wq1_sb, rhs=zqT_sb, start=True, stop=True)
            hq_sb = mid.tile([H, BL], BF16)
            nc.scalar.activation(
                out=hq_sb, in_=hq_ps, func=ACTF.Relu, bias=bq1_sb, scale=1.0
            )

            # ---- remaining decoder tiles: spread the leftover xe tiles
            # evenly across the xq stream so both streams (and both evict
            # engines) stay active until the very last tile ----
            n_early = xe_cur[0]
            n_rest = ND - n_early
            sent = 0
            for j in range(ND):
                target = (j + 1) * n_rest // ND
                while sent < target:
                    m = n_early + sent
                    xe_tile(m, use_act=(m % 2 == 1))
                    sent += 1
                xq_tile(j, use_act=(j % 2 == 0))

    nc.compile()
    return nc


def _pad_bias_t(b):
    """[D] bias -> [DT, ND] where column m is b[m*DT : m*DT+DT] (zero padded)."""
    bp = np.zeros(ND * DT, dtype=np.float32)
    bp[:D] = b
    return np.ascontiguousarray(bp.reshape(ND, DT).T)


def _split_bf16(a):
    """Exact-ish split: a ~= hi + lo with both bf16 (lo holds the residual)."""
    hi = a.astype(ml_dtypes.bfloat16)
    lo = (a - hi.astype(np.float32)).astype(ml_dtypes.bfloat16)
    return np.ascontiguousarray(hi), np.ascontiguousarray(lo)


def _tile_pm(a):
    """[Drows, C] -> zero-pad rows to ND*128 -> partition-major [128, ND, C]."""
    rows, c = a.shape
    out = np.zeros((ND * 128, c), dtype=a.dtype)
    out[:rows] = a
    return np.ascontiguousarray(out.reshape(ND, 128, c).transpose(1, 0, 2))


def _prep_shared(inputs):
    emb = np.asarray(inputs["embeddings"], dtype=np.float32)
    dl = np.concatenate(
        [-2.0 * emb.T, (emb * emb).sum(axis=1, dtype=np.float32)[None, :]], axis=0
    ).astype(np.float32)
    iota = np.tile(np.arange(K, dtype=np.float32), (128, NB))
    w1h, w1l = _split_bf16(np.asarray(inputs["enc_w1"], np.float32))
    shared = {
        "w1h": _tile_pm(w1h),
        "w1l": _tile_pm(w1l),
        "b1": np.asarray(inputs["enc_b1"], np.float32).reshape(H, 1),
        "w2": np.ascontiguousarray(inputs["enc_w2"], dtype=np.float32),
        "b2": np.asarray(inputs["enc_b2"], np.float32).reshape(Z, 1),
        "emb": np.ascontiguousarray(emb),
        "dl": np.ascontiguousarray(dl),
        "we1": np.ascontiguousarray(inputs["dec_e_w1"], dtype=np.float32),
        "be1": np.asarray(inputs["dec_e_b1"], np.float32).reshape(H, 1),
        "we2": np.ascontiguousarray(
            np.asarray(inputs["dec_e_w2"], np.float32).astype(ml_dtypes.bfloat16)
        ),
        "be2t": _pad_bias_t(np.asarray(inputs["dec_e_b2"], np.float32)),
        "wq1": np.ascontiguousarray(inputs["dec_q_w1"], dtype=np.float32),
        "bq1": np.asarray(inputs["dec_q_b1"], np.float32).reshape(H, 1),
        "wq2": np.ascontiguousarray(
            np.asarray(inputs["dec_q_w2"], np.float32).astype(ml_dtypes.bfloat16)
        ),
        "bq2t": _pad_bias_t(np.asarray(inputs["dec_q_b2"], np.float32)),
        "iota4": np.ascontiguousarray(iota),
        "desc4": np.ascontiguousarray(float(K - 1) - iota),
        "ident": np.eye(128, dtype=np.float32),
    }
    return shared


_NC_CACHE = None


def _run(inputs, trace=False, **kwargs):
    global _NC_CACHE
    if _NC_CACHE is None:
        _NC_CACHE = build_nc()
    nc = _NC_CACHE

    x = np.asarray(inputs["x"], dtype=np.float32)
    shared = _prep_shared(inputs)
    in_maps = []
    for c in range(NCORES):
        m = dict(shared)
        xt = np.ascontiguousarray(x[c * BL : (c + 1) * BL, :].T)
        xth, xtl = _split_bf16(xt)
        m["xh"], m["xl"] = _tile_pm(xth), _tile_pm(xtl)
        in_maps.append(m)

    try:
        res = run_bass_kernel_spmd(
            nc, in_maps, core_ids=list(range(NCORES)), trace=trace, **kwargs
        )
    except Exception:
        # transient NRT/device hiccups occasionally fail a run; retry once
        import time as _time

        _time.sleep(5)
        res = run_bass_kernel_spmd(
            nc, in_maps, core_ids=list(range(NCORES)), trace=trace, **kwargs
        )
    outs = res.results

    x_e = np.ascontiguousarray(
        np.concatenate([o["xeT"].T.astype(np.float32) for o in outs], axis=0), dtype=np.float32
    )
    x_q = np.ascontiguousarray(
        np.concatenate([o["xqT"].T.astype(np.float32) for o in outs], axis=0), dtype=np.float32
    )
    z_e = np.concatenate([o["ze"] for o in outs], axis=0)
    z_q = np.concatenate([o["zq"] for o in outs], axis=0)
    k = np.concatenate([o["ko"][:, 0] for o in outs], axis=0).astype(np.int32)
    z_dist = np.concatenate([o["zd"] for o in outs], axis=0)
    dist_prob = np.concatenate([o["dp"] for o in outs], axis=0)
    return (x_e, x_q, z_e, z_q, k, z_dist, dist_prob), res


def kernel(**inputs):
    out, _ = _run(inputs, trace=False)
    return out
